# revision 17
# baseline (speedup 1.0000x reference)
"""KeypointFlowLoss Trainium2 kernel.

The loss only reads each flow at the K keypoint pixels that the reference
scatters into the ground-truth flow image (every other pixel has gt == 0 and
mask == 0), so instead of streaming 5 x [16,2,512,512] f32 from HBM we gather
exactly the needed pixels with indirect DMA and reduce on-chip.

Sharding: data-parallel over the batch dim — core c owns batches
[2c, 2c+2). The five flows are stacked into one DRAM tensor per core so a
single indirect DMA gathers all 5 flows x 2 channels per keypoint (indirect
DMAs are gpsimd-only, so separate tensors would serialize five gathers on the
Pool queue). Each core emits [34, 6] per-keypoint partials ([5 masked EPEs,
mask]); the host all-reduces the partials and applies the weighted division.

Critical path: kps DMA (ACT queue) -> 2 fused int ops on DVE for gather
offsets -> one indirect gather (Pool) -> 4 DVE ops for the masked EPE
(pow(x, 0.5) fused with the mask multiply via scalar_tensor_tensor) ->
out DMA (SP). The constant offset-bias table rides a parallel DMA on the
idle SP queue; disp/mask are computed off-path during the gather window.
"""

import numpy as np

import concourse.bacc as bacc
import concourse.bass as bass
import concourse.mybir as mybir
import concourse.tile as tile
from concourse.bass import IndirectOffsetOnAxis
from concourse.bass_utils import run_bass_kernel_spmd

B, CH, H, W = 16, 2, 512, 512
K = 17
NF = 5
NCORES = 8
BL = B // NCORES          # batches per core
NP = BL * K               # keypoints per core
GAMMA = 0.8
LOSS_WEIGHT = 1.0

HW = H * W
CHW = CH * HW
FLAT = NF * BL * CHW      # elements in the per-core stacked flow tensor

F32 = mybir.dt.float32
I32 = mybir.dt.int32

_PROGRAM = None
_RUN_KWARGS = {}      # test harness can set {"trace": True} to profile
_LAST_RESULTS = None

# constant element-offset bias: cadd[p, f, c] = f*BL*CHW + (p >= K)*CHW + c*HW
_CADD = (
    np.arange(NF, dtype=np.int64)[None, :, None] * (BL * CHW)
    + (np.arange(NP, dtype=np.int64)[:, None, None] >= K) * CHW
    + np.arange(CH, dtype=np.int64)[None, None, :] * HW
).astype(np.int32).reshape(NP, NF * CH)


def _build_program():
    nc = bacc.Bacc(None, target_bir_lowering=False)

    flows = nc.dram_tensor("flows", [NF, BL, CH, H, W], F32, kind="ExternalInput")
    # host pre-arranges kps as [NP, 14] rows of [x0, y0, x1, y1, cadd*10]
    # where cadd is the constant offset-bias table (shape-derived, not data)
    kps = nc.dram_tensor("kps", [NP, 4 + NF * CH], I32, kind="ExternalInput")
    out = nc.dram_tensor("out", [NP, NF + 1], F32, kind="ExternalOutput")

    with tile.TileContext(nc) as tc:
        with tc.tile_pool(name="sbuf", bufs=1) as sb:
            # single input DMA: keypoints + constant table in one tile.
            # SP queue: the ACT queue's Sqrt table load would delay it.
            kt = sb.tile([NP, 4 + NF * CH], I32)
            nc.sync.dma_start(out=kt[:], in_=kps[:])
            cadd = kt[:, 4:4 + NF * CH].rearrange("p (f c) -> p f c", c=CH)

            # ---- critical path: element offsets y*W + (x + cadd) ----
            xc = sb.tile([NP, NF, CH], I32)
            nc.vector.tensor_tensor(
                out=xc[:],
                in0=kt[:, 0:1].unsqueeze(2).broadcast_to([NP, NF, CH]),
                in1=cadd, op=mybir.AluOpType.add)
            offs = sb.tile([NP, NF, CH], I32)
            nc.vector.scalar_tensor_tensor(
                out=offs[:],
                in0=kt[:, 1:2].unsqueeze(2).broadcast_to([NP, NF, CH]),
                scalar=W, in1=xc[:],
                op0=mybir.AluOpType.mult,
                op1=mybir.AluOpType.add)

            # ---- single gather: all 5 flows x 2 channels per keypoint ----
            g = sb.tile([NP, NF, CH], F32)
            flat = bass.AP(flows, 0, [[1, FLAT], [1, 1]])
            nc.gpsimd.indirect_dma_start(
                out=g[:], out_offset=None, in_=flat,
                in_offset=IndirectOffsetOnAxis(ap=offs[:], axis=0))

            # ---- off-path during the gather: disp and mask ----
            kf = sb.tile([NP, 4], F32)
            nc.vector.tensor_copy(out=kf[:], in_=kt[:, 0:4])  # int -> float, exact
            disp = sb.tile([NP, 2], F32)
            nc.vector.tensor_tensor(out=disp[:], in0=kf[:, 2:4], in1=kf[:, 0:2],
                                    op=mybir.AluOpType.subtract)
            dsq0 = sb.tile([NP, 2], F32)
            nc.vector.tensor_tensor(out=dsq0[:], in0=disp[:], in1=disp[:],
                                    op=mybir.AluOpType.mult)
            r2 = sb.tile([NP, 1], F32)
            nc.vector.tensor_tensor(out=r2[:], in0=dsq0[:, 0:1], in1=dsq0[:, 1:2],
                                    op=mybir.AluOpType.add)
            # vcols = [5 masked EPE columns, mask]; single writer (DVE) so the
            # out DMA needs only one semaphore. mask written first (off-path).
            vcols = sb.tile([NP, NF + 1], F32)
            nc.vector.tensor_scalar(out=vcols[:, NF:NF + 1], in0=r2[:],
                                    scalar1=0.0, scalar2=None,
                                    op0=mybir.AluOpType.is_gt)

            # ---- post-gather: masked EPE, pow(x,0.5)*mask fused ----
            d = sb.tile([NP, NF, CH], F32)
            nc.vector.tensor_tensor(
                out=d[:], in0=g[:],
                in1=disp[:].unsqueeze(1).broadcast_to([NP, NF, CH]),
                op=mybir.AluOpType.subtract)
            nc.vector.tensor_tensor(out=d[:], in0=d[:], in1=d[:],
                                    op=mybir.AluOpType.mult)
            s = sb.tile([NP, NF], F32)
            nc.vector.tensor_reduce(out=s[:].unsqueeze(2), in_=d[:],
                                    op=mybir.AluOpType.add,
                                    axis=mybir.AxisListType.X)
            # sqrt(s * mask) == sqrt(s) * mask for mask in {0, 1}; the mask
            # multiply rides the activation's per-partition scale input
            nc.scalar.activation(out=vcols[:, 0:NF], in_=s[:],
                                 func=mybir.ActivationFunctionType.Sqrt,
                                 scale=vcols[:, NF:NF + 1])

            nc.scalar.dma_start(out=out[:], in_=vcols[:])

    nc.finalize()
    return nc


def _get_program():
    global _PROGRAM
    if _PROGRAM is None:
        _PROGRAM = _build_program()
    return _PROGRAM


def kernel(**inputs):
    flows = [np.asarray(inputs[f"flow{i}"], dtype=np.float32) for i in range(NF)]
    kps = np.ascontiguousarray(np.asarray(inputs["kps"], dtype=np.int32))

    nc = _get_program()

    in_maps = []
    for c in range(NCORES):
        sl = slice(c * BL, (c + 1) * BL)
        # [BL,2,K,2] -> [BL,K,2,2] -> [NP,4] rows of [x0,y0,x1,y1],
        # then append the constant offset-bias table -> [NP,14]
        kps_r = np.concatenate(
            [kps[sl].transpose(0, 2, 1, 3).reshape(NP, 4), _CADD], axis=1)
        in_maps.append({
            "flows": np.stack([flows[i][sl] for i in range(NF)]),
            "kps": np.ascontiguousarray(kps_r),
        })

    results = run_bass_kernel_spmd(nc, in_maps, core_ids=list(range(NCORES)),
                                   **_RUN_KWARGS)
    globals()["_LAST_RESULTS"] = results

    # all-reduce the per-keypoint partials: [NCORES*NP, 6]
    total = np.zeros(NF + 1, dtype=np.float32)
    for r in results.results:
        total += r["out"].reshape(NP, NF + 1).astype(np.float32).sum(axis=0)

    sums, cnt = total[:NF], total[NF]
    weights = (np.float32(GAMMA) ** np.arange(NF - 1, -1, -1, dtype=np.float32))
    means = sums / np.float32(cnt)
    loss = np.float32(np.sum(weights * means, dtype=np.float32) * np.float32(LOSS_WEIGHT))
    return np.asarray(loss, dtype=np.float32)
